# revision 14
# baseline (speedup 1.0000x reference)
"""Trainium2 Bass kernel for the DCE (dynamic contrast-enhanced) 2CXM signal model.

Algorithmic core (replaces the 640-step FFT convolution of the reference):

  The sampled convolution response is, per pixel, p_k(theta) =
  sum_t A[k,t] e^{-0.1 t theta} evaluated at theta_m / theta_p -- a Laplace-
  type function of a single scalar.  It is approximated to ~1e-6 relative
  (vs a 2e-2 tolerance) by a J=32 sum of exponentials
      p_k(theta) ~= sum_j B[k,j] e^{-alpha_j theta}
  with alpha_0 = 0 and alpha_1..31 geometrically spaced; B is fitted on the
  host by ridge least squares over theta in [0.02, 64] (the attainable range
  for param in [0.05, 1]^4 is well inside).  conc = c1*p(theta_m) +
  c2*p(theta_p) with the same per-pixel c1/c2 closed forms as before.

  The SPGR epilogue uses the exact identity
      1/(1 - c e^{-u}) = (coth(u/2) + 1)/2,  u = TR*(R1 + R1CA*conc) - ln c
  so sig = (K2/2)/tanh(u/2) + (K1 + K2/2): one Tanh, one reciprocal, one
  affine.  Exp and Tanh share one ACT table set (no in-loop table loads).

Device layout (per core, 12800 pixels):
  - prep in pixel-partition layout [128, 100] (pixel = p*100 + q), with the
    thm|thp and c1|c2 quantities packed as column-halves of [128, 200] tiles
    so most chain ops run once at free-size 200; ~25 ops total.  theta rows
    ship to a rows4 [4, 6400] SBUF tensor (2 reshape DMAs), c rows (fp16) to
    DRAM for broadcast.
  - main loop over 7 pixel pairs (6 x 1024 + 256), 4-way stacked tiles
    [128 = 4 blocks x 32 alphas, Wp]: blocks (em h0, em h1, ep h0, ep h1).
    PE ones-matmul broadcasts theta rows -> PSUM, ACT computes the exp basis
    in ONE fp16 instruction per pair (per-partition scale = -alpha), a DMA
    broadcasts fp16 c rows, DVE multiplies at 2x fp16 throughput, and ONE
    fp16 matmul against the block lhsT B4 [128, 100] contracts basis -> conc
    for both halves at once ([100, Wp] PSUM: rows 0-49 half0, 50-99 half1).
  - epilogue: ACT Tanh, DVE reciprocal, Pool affine -> fp16, DMA out per pair.
"""

import os

import numpy as np

H = W = 320
NPIX = H * W
NCORES = 8
SHARD = NPIX // NCORES      # 12800 pixels per core
HALF = SHARD // 2           # 6400 (stacking half)
QP = 100                    # free size of the [128, 100] prep layout
PW = 1024                   # pixels per half per pair (2 PSUM banks)
NP = (HALF + PW - 1) // PW  # 7 pairs (6 x 1024 + 1 x 256)
J = 32                      # exponential-basis size
L = 589                     # fine time-grid length
TS = 50                     # output time samples
STEP = 0.1
DELAY = 30                  # 3s bolus delay in fine-grid samples

# SPGR constants (from reference.py)
SIG_BASELINE = 100.0
R1 = 1.0
R1CA = 4.3
FA = 10.0
TR = 0.00487

_CACHE: dict = {}


def _spgr_consts():
    fa = FA * np.pi / 180.0
    cosf = float(np.cos(np.float32(fa)))
    sinf = float(np.sin(np.float32(fa)))
    E1 = float(np.exp(np.float32(-TR * R1)))
    M0 = SIG_BASELINE * (1.0 - cosf * E1) / (sinf * (1.0 - E1))
    M0t = M0 * sinf
    M_steady = M0t * (1.0 - E1) / (1.0 - E1 * cosf)
    C0 = SIG_BASELINE - M_steady
    K1 = M0t / cosf + C0
    K2 = -M0t * (1.0 - cosf) / cosf
    a = TR * R1CA
    b = TR * R1 - float(np.log(cosf))
    return K1, K2, a, b


def _alphas():
    return np.concatenate(
        [[0.0], np.geomspace(0.05, 58.8, J - 1)]
    ).astype(np.float64)


def _build_bass():
    import concourse.bass as bass
    import concourse.tile as tile
    from concourse import bacc, mybir
    from contextlib import ExitStack

    f32 = mybir.dt.float32
    f32r = mybir.dt.float32r
    f16 = mybir.dt.float16
    AF = mybir.ActivationFunctionType
    ALU = mybir.AluOpType

    K1, K2, a_, b_ = _spgr_consts()

    nc = bacc.Bacc()
    pmap = nc.dram_tensor("pmap", [4, SHARD], f32, kind="ExternalInput")
    b4d = nc.dram_tensor("b4", [128, 2 * TS], f16, kind="ExternalInput")
    ones4d = nc.dram_tensor("ones4", [4, 128], f32, kind="ExternalInput")
    svd = nc.dram_tensor("sv", [128, 1], f32, kind="ExternalInput")
    sig2 = nc.dram_tensor("sig2", [2 * TS, HALF], f16, kind="ExternalOutput")

    with tile.TileContext(nc) as tc, ExitStack() as ctx:
        const = ctx.enter_context(tc.tile_pool(name="const", bufs=1))
        rows = ctx.enter_context(tc.tile_pool(name="rows", bufs=1))
        ebp = ctx.enter_context(tc.tile_pool(name="ebp", bufs=7))
        cbp = ctx.enter_context(tc.tile_pool(name="cbp", bufs=4))
        rhp = ctx.enter_context(tc.tile_pool(name="rhp", bufs=3))
        ep1 = ctx.enter_context(tc.tile_pool(name="ep1", bufs=2))
        ep2 = ctx.enter_context(tc.tile_pool(name="ep2", bufs=2))
        obp = ctx.enter_context(tc.tile_pool(name="obp", bufs=3))
        psbc = ctx.enter_context(
            tc.tile_pool(name="psbc", bufs=2, space=bass.MemorySpace.PSUM)
        )
        pcc = ctx.enter_context(
            tc.tile_pool(name="pcc", bufs=2, space=bass.MemorySpace.PSUM)
        )
        prep = ctx.enter_context(tc.tile_pool(name="prep", bufs=1))
        dpool = ctx.enter_context(tc.tile_pool(name="drows", bufs=1, space="DRAM"))

        V = nc.vector
        G = nc.gpsimd

        # ---- constants ----
        b4_sb = const.tile([128, 2 * TS], f16, tag="b4_sb", name="b4_sb")
        o4_sb = const.tile([4, 128], f32, tag="o4_sb", name="o4_sb")
        o4_r = const.tile([4, 128], f32r, tag="o4_r", name="o4_r")
        sv_sb = const.tile([128, 1], f32, tag="sv_sb", name="sv_sb")
        tb_sb = const.tile([2 * TS, 1], f32, tag="tb_sb", name="tb_sb")
        nc.scalar.dma_start(out=b4_sb[:], in_=b4d[:])
        nc.scalar.dma_start(out=o4_sb[:], in_=ones4d[:])
        nc.scalar.dma_start(out=sv_sb[:], in_=svd[:])
        G.tensor_copy(o4_r[:], o4_sb[:])
        G.memset(tb_sb, float(b_ / 2.0))

        # ---- per-pixel prep, [128, 100] pixel-partition layout ----
        def ptile(tag, w=QP):
            return prep.tile([128, w], f32, tag=tag, name=tag)

        pin1 = prep.tile([128, 2, QP], f32, tag="pin1", name="pin1")
        pin2 = prep.tile([128, 2, QP], f32, tag="pin2", name="pin2")
        # pin1 = (fp, ps), pin2 = (ve, vp): chain can start after pin1 lands
        nc.sync.dma_start(
            out=pin1, in_=pmap[2:4, :].rearrange("v (p q) -> p v q", p=128)
        )
        nc.sync.dma_start(
            out=pin2, in_=pmap[0:2, :].rearrange("v (p q) -> p v q", p=128)
        )
        fp = pin1[:, 0, :]; ps = pin1[:, 1, :]
        ve = pin2[:, 0, :]; vp = pin2[:, 1, :]
        pin1f = pin1.rearrange("p v q -> p (v q)")

        thmthp = ptile("thmthp", 2 * QP)
        c1c2 = prep.tile([128, 2 * QP], f16, tag="c1c2", name="c1c2")
        thm = thmthp[:, 0:QP]
        thp = thmthp[:, QP : 2 * QP]

        # critical chain (DVE unless noted)
        rfps = ptile("rfps", 2 * QP)
        V.reciprocal_approx_fast(rfps, pin1f)              # (1/fp, 1/ps)
        rfp = rfps[:, 0:QP]; rps = rfps[:, QP : 2 * QP]
        Te = ptile("Te"); V.tensor_mul(Te, ve, rps)
        sv2 = ptile("sv2"); V.tensor_add(sv2, vp, ve)
        T_ = ptile("T_"); V.tensor_mul(T_, sv2, rfp)
        s_ = ptile("s_"); V.tensor_add(s_, T_, Te)
        Tc = ptile("Tc"); G.tensor_mul(Tc, vp, rfp)        # Pool, off-chain
        q4 = ptile("q4")
        V.scalar_tensor_tensor(q4, Tc, 4.0, Te, op0=ALU.mult, op1=ALU.mult)
        sq = ptile("sq"); V.tensor_mul(sq, s_, s_)
        V.tensor_sub(sq, sq, q4)
        d_ = ptile("d_"); nc.scalar.sqrt(d_, sq)           # ACT (sqrt table)
        denq = ptile("denq", 2 * QP)
        den = denq[:, 0:QP]
        V.tensor_add(den, s_, d_)
        G.tensor_copy(denq[:, QP : 2 * QP], q4)            # Pool, off-chain
        rr1 = ptile("rr1", 2 * QP)
        V.reciprocal_approx_fast(rr1, denq)                # (1/den, 1/q4)
        V.tensor_scalar_mul(thm, rr1[:, 0:QP], 2.0)
        V.scalar_tensor_tensor(
            thp, den, 2.0, rr1[:, QP : 2 * QP], op0=ALU.mult, op1=ALU.mult
        )

        # theta rows ship as soon as thm/thp are written
        rowsT = rows.tile([4, HALF], f32, tag="rowsT", name="rowsT")
        nc.sync.dma_start(out=rowsT[0:2, :], in_=thmthp[:, 0:QP])
        nc.sync.dma_start(out=rowsT[2:4, :], in_=thmthp[:, QP : 2 * QP])

        # geometric-sum normalizers Sm/Sp, packed at free-size 200
        e1 = ptile("e1", 2 * QP)
        nc.scalar.activation(e1, thmthp, AF.Exp, bias=0.0, scale=-STEP)
        eL = ptile("eL", 2 * QP)
        nc.scalar.activation(eL, thmthp, AF.Exp, bias=0.0, scale=-STEP * L)
        Te2 = ptile("Te2", 2 * QP)                          # (Te, Te), off-chain
        G.tensor_copy(Te2[:, 0:QP], Te)
        G.tensor_copy(Te2[:, QP : 2 * QP], Te)
        umup = ptile("umup", 2 * QP)
        V.tensor_mul(umup, Te2, thmthp)                     # (Te*thm, Te*thp)
        albe = ptile("albe", 2 * QP)
        G.tensor_scalar(albe[:, 0:QP], umup[:, 0:QP], -1.0, 1.0,
                        op0=ALU.mult, op1=ALU.add)          # alp = 1 - Te*thm
        G.tensor_scalar_sub(albe[:, QP : 2 * QP], umup[:, QP : 2 * QP], 1.0)
        den1 = ptile("den1", 2 * QP)
        V.tensor_scalar(den1, e1, -1.0, 1.0, op0=ALU.mult, op1=ALU.add)
        V.reciprocal_approx_fast(den1, den1)                # 1/(1-e1)
        numL = ptile("numL", 2 * QP)
        G.tensor_scalar(numL, eL, -1.0, 1.0, op0=ALU.mult, op1=ALU.add)
        SmSp = ptile("SmSp", 2 * QP)
        V.tensor_mul(SmSp, numL, den1)                      # (Sm, Sp)
        Sm = SmSp[:, 0:QP]; Sp = SmSp[:, QP : 2 * QP]
        asbs = ptile("asbs", 2 * QP)
        V.tensor_mul(asbs, albe, SmSp)                      # (alp*Sm, bet*Sp)
        dsab = ptile("dsab", 2 * QP)
        V.tensor_sub(dsab[:, 0:QP], Sm, Sp)
        V.tensor_add(dsab[:, QP : 2 * QP], asbs[:, 0:QP], asbs[:, QP : 2 * QP])
        rr2 = ptile("rr2", 2 * QP)
        V.reciprocal_approx_fast(rr2, dsab)                 # (1/dS, 1/ab)
        rdS = rr2[:, 0:QP]; rab = rr2[:, QP : 2 * QP]
        w1 = ptile("w1"); G.tensor_mul(w1, vp, albe[:, 0:QP])     # off-chain
        w2 = ptile("w2"); G.tensor_mul(w2, vp, albe[:, QP : 2 * QP])
        vede = ptile("vede"); V.tensor_mul(vede, ve, rdS)
        w1b = ptile("w1b"); V.tensor_mul(w1b, w1, rab)
        w2b = ptile("w2b"); V.tensor_mul(w2b, w2, rab)
        V.tensor_add(c1c2[:, 0:QP], w1b, vede)              # c1 (fp16)
        V.tensor_sub(c1c2[:, QP : 2 * QP], w2b, vede)       # c2 (fp16)

        # c rows to DRAM (fp16) for per-pair broadcast DMAs
        crows_d = dpool.tile([4, HALF], f16, tag="crows_d", name="crows_d")
        nc.gpsimd.dma_start(out=crows_d[0:2, :], in_=c1c2[:, 0:QP])
        nc.gpsimd.dma_start(out=crows_d[2:4, :], in_=c1c2[:, QP : 2 * QP])

        # ---- main loop over pixel pairs ----
        tanh_scale = float(a_ / 2.0)
        fs1 = float(K2 / 2.0)
        fs2 = float(K1 + K2 / 2.0)

        for p in range(NP):
            lo = p * PW
            Wp = min(PW, HALF - lo)
            sl = slice(lo, lo + Wp)

            th_bc = psbc.tile([128, PW], f32, tag="th_bc", name="th_bc")
            for mlo in range(0, Wp, 512):
                mw = min(512, Wp - mlo)
                nc.tensor.matmul(
                    th_bc[:, mlo : mlo + mw], o4_r[:],
                    rowsT[:, lo + mlo : lo + mlo + mw].bitcast(f32r),
                    start=True, stop=True,
                )
            eb = ebp.tile([128, PW], f16, tag="eb", name="eb")
            nc.scalar.activation(
                eb[:, :Wp], th_bc[:, :Wp], AF.Exp, bias=0.0, scale=sv_sb[:, 0:1]
            )
            cb = cbp.tile([128, PW], f16, tag="cb", name="cb")
            c0 = crows_d[0, lo : lo + Wp]
            nc.scalar.dma_start(
                out=cb[:, :Wp],
                in_=bass.AP(
                    tensor=c0.tensor, offset=c0.offset,
                    ap=[[HALF, 4], [0, 32], [1, Wp]],
                ),
            )
            rhs = rhp.tile([128, PW], f16, tag="rhs", name="rhs")
            V.tensor_mul(rhs[:, :Wp], eb[:, :Wp], cb[:, :Wp])

            conc = pcc.tile([2 * TS, PW], f32, tag="conc", name="conc")
            for mlo in range(0, Wp, 512):
                mw = min(512, Wp - mlo)
                nc.tensor.matmul(
                    conc[:, mlo : mlo + mw], b4_sb[:],
                    rhs[:, mlo : mlo + mw], start=True, stop=True,
                )

            tht = ep1.tile([2 * TS, PW], f32, tag="tht", name="tht")
            nc.scalar.activation(
                tht[:, :Wp], conc[:, :Wp], AF.Tanh,
                bias=tb_sb, scale=tanh_scale,
            )
            rt = ep2.tile([2 * TS, PW], f32, tag="rt", name="rt")
            V.reciprocal_approx_fast(rt[:, :Wp], tht[:, :Wp])
            ob = obp.tile([2 * TS, PW], f16, tag="ob", name="ob")
            G.tensor_scalar(
                ob[:, :Wp], rt[:, :Wp], fs1, fs2, op0=ALU.mult, op1=ALU.add
            )
            nc.sync.dma_start(out=sig2[:, sl], in_=ob[:, :Wp])

    nc.compile()
    return nc


def _host_prep(sample_time: np.ndarray, Cp: np.ndarray):
    """Build the AIF response matrix A, fit the J-term exponential basis, and
    pack the block lhsT / broadcast-ones / scale constants."""
    t_end = float(np.asarray(sample_time)[-1])
    Lf = int(round(t_end / STEP)) + 1
    t_samp = np.arange(Lf, dtype=np.float32) * np.float32(STEP)
    aifci = np.interp(
        t_samp.astype(np.float64),
        np.asarray(sample_time, np.float64),
        np.asarray(Cp, np.float64),
    ).astype(np.float32)
    aif = np.concatenate([np.zeros(DELAY, np.float32), aifci[:-DELAY]])
    idx = np.searchsorted(t_samp, np.asarray(sample_time, np.float32), side="left")
    idx = np.minimum(idx, Lf - 1)

    # A[k, t] = aif[idx[k] - t] for t <= idx[k] (conv 'full' sampled at idx)
    A = np.zeros((TS, 640), np.float64)
    for k in range(TS):
        i = int(idx[k])
        A[k, : i + 1] = aif[i::-1]

    alphas = _alphas()
    tg = np.arange(640, dtype=np.float64)
    g = np.geomspace(0.02, 64.0, 4000)
    P = np.exp(-0.1 * np.outer(g, tg)) @ A.T          # [G, 50]
    M = np.exp(-np.outer(g, alphas))                  # [G, J]
    B = np.linalg.solve(M.T @ M + 1e-8 * np.eye(J), M.T @ P)  # [J, 50]
    B = B.astype(np.float32)

    # block lhsT [128, 100]: blocks (em h0, em h1, ep h0, ep h1) x 32 alphas
    # (rows tensors are (thm h0, thm h1, thp h0, thp h1) so that each rows
    # quantity ships in one [128,100] -> [2,6400] reshape DMA)
    b4 = np.zeros((128, 2 * TS), np.float32)
    b4[0:J, 0:TS] = B
    b4[J : 2 * J, TS : 2 * TS] = B
    b4[2 * J : 3 * J, 0:TS] = B
    b4[3 * J : 4 * J, TS : 2 * TS] = B

    ones4 = np.zeros((4, 128), np.float32)
    for r in range(4):
        ones4[r, r * J : (r + 1) * J] = 1.0

    sv = (-alphas[np.arange(128) % J]).reshape(128, 1).astype(np.float32)
    return b4.astype(np.float16), ones4, sv


def kernel(param: np.ndarray, sample_time: np.ndarray, Cp: np.ndarray) -> np.ndarray:
    from concourse.bass_utils import run_bass_kernel_spmd

    if "nc" not in _CACHE:
        _CACHE["nc"] = _build_bass()
    nc = _CACHE["nc"]

    b4, ones4, sv = _host_prep(sample_time, Cp)
    pflat = np.ascontiguousarray(np.asarray(param, np.float32).reshape(4, NPIX))
    in_maps = []
    for c in range(NCORES):
        in_maps.append(
            {
                "pmap": np.ascontiguousarray(pflat[:, c * SHARD : (c + 1) * SHARD]),
                "b4": b4,
                "ones4": ones4,
                "sv": sv,
            }
        )
    res = run_bass_kernel_spmd(
        nc,
        in_maps,
        core_ids=list(range(NCORES)),
        trace=bool(int(os.environ.get("DCE_TRACE", "0"))),
    )
    if res.exec_time_ns is not None:
        _CACHE["exec_time_ns"] = res.exec_time_ns
    # sig2 [100, 6400] fp16 per core: rows 0-49 half0 pixels, rows 50-99 half1
    parts = []
    for r in res.results:
        s2 = np.asarray(r["sig2"], np.float32)
        parts.append(np.concatenate([s2[:TS, :], s2[TS:, :]], axis=1))
    out = np.concatenate(parts, axis=1)
    return out.reshape(TS, 1, H, W)


# revision 15
# speedup vs baseline: 1.2592x; 1.2592x over previous
"""Trainium2 Bass kernel for the DCE (dynamic contrast-enhanced) 2CXM signal model.

Algorithmic core (replaces the 640-step FFT convolution of the reference):

  The sampled convolution response is, per pixel, p_k(theta) =
  sum_t A[k,t] e^{-0.1 t theta} evaluated at theta_m / theta_p -- a Laplace-
  type function of a single scalar.  It is approximated to ~1e-6 relative
  (vs a 2e-2 tolerance) by a J=32 sum of exponentials
      p_k(theta) ~= sum_j B[k,j] e^{-alpha_j theta}
  with alpha_0 = 0 and alpha_1..31 geometrically spaced; B is fitted on the
  host by ridge least squares over theta in [0.02, 64] (the attainable range
  for param in [0.05, 1]^4 is well inside).  conc = c1*p(theta_m) +
  c2*p(theta_p) with the same per-pixel c1/c2 closed forms as before.

  The SPGR epilogue uses the exact identity
      1/(1 - c e^{-u}) = (coth(u/2) + 1)/2,  u = TR*(R1 + R1CA*conc) - ln c
  so sig = (K2/2)/tanh(u/2) + (K1 + K2/2): one Tanh, one reciprocal, one
  affine.  Exp and Tanh share one ACT table set (no in-loop table loads).

Device layout (per core, 12800 pixels):
  - prep in pixel-partition layout [128, 100] (pixel = p*100 + q), with the
    thm|thp and c1|c2 quantities packed as column-halves of [128, 200] tiles
    so most chain ops run once at free-size 200; ~25 ops total.  theta rows
    ship to a rows4 [4, 6400] SBUF tensor (2 reshape DMAs), c rows (fp16) to
    DRAM for broadcast.
  - main loop over 7 pixel pairs (6 x 1024 + 256), 4-way stacked tiles
    [128 = 4 blocks x 32 alphas, Wp]: blocks (em h0, em h1, ep h0, ep h1).
    PE ones-matmul broadcasts theta rows -> PSUM, ACT computes the exp basis
    in ONE fp16 instruction per pair (per-partition scale = -alpha), a DMA
    broadcasts fp16 c rows, DVE multiplies at 2x fp16 throughput, and ONE
    fp16 matmul against the block lhsT B4 [128, 100] contracts basis -> conc
    for both halves at once ([100, Wp] PSUM: rows 0-49 half0, 50-99 half1).
  - epilogue: ACT Tanh, DVE reciprocal, Pool affine -> fp16, DMA out per pair.
"""

import os

import numpy as np

H = W = 320
NPIX = H * W
NCORES = 8
SHARD = NPIX // NCORES      # 12800 pixels per core
HALF = SHARD // 2           # 6400 (stacking half)
QP = 100                    # free size of the [128, 100] prep layout
PW = 1024                   # pixels per half per pair (2 PSUM banks)
NP = (HALF + PW - 1) // PW  # 7 pairs (6 x 1024 + 1 x 256)
J = 32                      # exponential-basis size
L = 589                     # fine time-grid length
TS = 50                     # output time samples
STEP = 0.1
DELAY = 30                  # 3s bolus delay in fine-grid samples

# SPGR constants (from reference.py)
SIG_BASELINE = 100.0
R1 = 1.0
R1CA = 4.3
FA = 10.0
TR = 0.00487

_CACHE: dict = {}


def _spgr_consts():
    fa = FA * np.pi / 180.0
    cosf = float(np.cos(np.float32(fa)))
    sinf = float(np.sin(np.float32(fa)))
    E1 = float(np.exp(np.float32(-TR * R1)))
    M0 = SIG_BASELINE * (1.0 - cosf * E1) / (sinf * (1.0 - E1))
    M0t = M0 * sinf
    M_steady = M0t * (1.0 - E1) / (1.0 - E1 * cosf)
    C0 = SIG_BASELINE - M_steady
    K1 = M0t / cosf + C0
    K2 = -M0t * (1.0 - cosf) / cosf
    a = TR * R1CA
    b = TR * R1 - float(np.log(cosf))
    return K1, K2, a, b


def _alphas():
    return np.concatenate(
        [[0.0], np.geomspace(0.05, 58.8, J - 1)]
    ).astype(np.float64)


def _build_bass():
    import concourse.bass as bass
    import concourse.tile as tile
    from concourse import bacc, mybir
    from contextlib import ExitStack

    f32 = mybir.dt.float32
    f32r = mybir.dt.float32r
    f16 = mybir.dt.float16
    AF = mybir.ActivationFunctionType
    ALU = mybir.AluOpType

    K1, K2, a_, b_ = _spgr_consts()

    nc = bacc.Bacc()
    pmap = nc.dram_tensor("pmap", [4, SHARD], f32, kind="ExternalInput")
    b4d = nc.dram_tensor("b4", [128, 2 * TS], f16, kind="ExternalInput")
    ones4d = nc.dram_tensor("ones4", [4, 128], f32, kind="ExternalInput")
    svd = nc.dram_tensor("sv", [128, 1], f32, kind="ExternalInput")
    sig2 = nc.dram_tensor("sig2", [2 * TS, HALF], f16, kind="ExternalOutput")

    with tile.TileContext(nc) as tc, ExitStack() as ctx:
        const = ctx.enter_context(tc.tile_pool(name="const", bufs=1))
        rows = ctx.enter_context(tc.tile_pool(name="rows", bufs=1))
        ebp = ctx.enter_context(tc.tile_pool(name="ebp", bufs=7))
        cbp = ctx.enter_context(tc.tile_pool(name="cbp", bufs=4))
        rhp = ctx.enter_context(tc.tile_pool(name="rhp", bufs=3))
        ep1 = ctx.enter_context(tc.tile_pool(name="ep1", bufs=2))
        ep2 = ctx.enter_context(tc.tile_pool(name="ep2", bufs=2))
        obp = ctx.enter_context(tc.tile_pool(name="obp", bufs=3))
        psbc = ctx.enter_context(
            tc.tile_pool(name="psbc", bufs=2, space=bass.MemorySpace.PSUM)
        )
        pcc = ctx.enter_context(
            tc.tile_pool(name="pcc", bufs=2, space=bass.MemorySpace.PSUM)
        )
        prep = ctx.enter_context(tc.tile_pool(name="prep", bufs=1))
        dpool = ctx.enter_context(tc.tile_pool(name="drows", bufs=1, space="DRAM"))

        V = nc.vector
        G = nc.gpsimd

        # ---- constants ----
        b4_sb = const.tile([128, 2 * TS], f16, tag="b4_sb", name="b4_sb")
        o4_sb = const.tile([4, 128], f32, tag="o4_sb", name="o4_sb")
        o4_r = const.tile([4, 128], f32r, tag="o4_r", name="o4_r")
        sv_sb = const.tile([128, 1], f32, tag="sv_sb", name="sv_sb")
        tb_sb = const.tile([2 * TS, 1], f32, tag="tb_sb", name="tb_sb")
        nc.scalar.dma_start(out=b4_sb[:], in_=b4d[:])
        nc.scalar.dma_start(out=o4_sb[:], in_=ones4d[:])
        nc.scalar.dma_start(out=sv_sb[:], in_=svd[:])
        G.tensor_copy(o4_r[:], o4_sb[:])
        G.memset(tb_sb, float(b_ / 2.0))

        # ---- per-pixel prep, [128, 100] pixel-partition layout ----
        def ptile(tag, w=QP):
            return prep.tile([128, w], f32, tag=tag, name=tag)

        pin1 = prep.tile([128, 2, QP], f32, tag="pin1", name="pin1")
        pin2 = prep.tile([128, 2, QP], f32, tag="pin2", name="pin2")
        # pin1 = (fp, ps), pin2 = (ve, vp): chain can start after pin1 lands
        nc.sync.dma_start(
            out=pin1, in_=pmap[2:4, :].rearrange("v (p q) -> p v q", p=128)
        )
        nc.sync.dma_start(
            out=pin2, in_=pmap[0:2, :].rearrange("v (p q) -> p v q", p=128)
        )
        fp = pin1[:, 0, :]; ps = pin1[:, 1, :]
        ve = pin2[:, 0, :]; vp = pin2[:, 1, :]
        pin1f = pin1.rearrange("p v q -> p (v q)")

        thmthp = ptile("thmthp", 2 * QP)
        c1c2 = prep.tile([128, 2 * QP], f16, tag="c1c2", name="c1c2")
        thm = thmthp[:, 0:QP]
        thp = thmthp[:, QP : 2 * QP]

        # critical chain (DVE unless noted)
        rfps = ptile("rfps", 2 * QP)
        V.reciprocal_approx_fast(rfps, pin1f)              # (1/fp, 1/ps)
        rfp = rfps[:, 0:QP]; rps = rfps[:, QP : 2 * QP]
        Te = ptile("Te"); V.tensor_mul(Te, ve, rps)
        sv2 = ptile("sv2"); V.tensor_add(sv2, vp, ve)
        T_ = ptile("T_"); V.tensor_mul(T_, sv2, rfp)
        s_ = ptile("s_"); V.tensor_add(s_, T_, Te)
        Tc = ptile("Tc"); G.tensor_mul(Tc, vp, rfp)        # Pool, off-chain
        q4 = ptile("q4")
        V.scalar_tensor_tensor(q4, Tc, 4.0, Te, op0=ALU.mult, op1=ALU.mult)
        sq = ptile("sq"); V.tensor_mul(sq, s_, s_)
        V.tensor_sub(sq, sq, q4)
        d_ = ptile("d_"); nc.scalar.sqrt(d_, sq)           # ACT (sqrt table)
        denq = ptile("denq", 2 * QP)
        den = denq[:, 0:QP]
        V.tensor_add(den, s_, d_)
        G.tensor_copy(denq[:, QP : 2 * QP], q4)            # Pool, off-chain
        rr1 = ptile("rr1", 2 * QP)
        V.reciprocal_approx_fast(rr1, denq)                # (1/den, 1/q4)
        V.tensor_scalar_mul(thm, rr1[:, 0:QP], 2.0)
        V.scalar_tensor_tensor(
            thp, den, 2.0, rr1[:, QP : 2 * QP], op0=ALU.mult, op1=ALU.mult
        )

        # theta rows ship as soon as thm/thp are written
        rowsT = rows.tile([4, HALF], f32, tag="rowsT", name="rowsT")
        nc.sync.dma_start(out=rowsT[0:2, :], in_=thmthp[:, 0:QP])
        nc.sync.dma_start(out=rowsT[2:4, :], in_=thmthp[:, QP : 2 * QP])

        # geometric-sum normalizers Sm/Sp, packed at free-size 200
        e1 = ptile("e1", 2 * QP)
        nc.scalar.activation(e1, thmthp, AF.Exp, bias=0.0, scale=-STEP)
        eL = ptile("eL", 2 * QP)
        nc.scalar.activation(eL, thmthp, AF.Exp, bias=0.0, scale=-STEP * L)
        Te2 = ptile("Te2", 2 * QP)                          # (Te, Te), off-chain
        G.tensor_copy(Te2[:, 0:QP], Te)
        G.tensor_copy(Te2[:, QP : 2 * QP], Te)
        umup = ptile("umup", 2 * QP)
        V.tensor_mul(umup, Te2, thmthp)                     # (Te*thm, Te*thp)
        albe = ptile("albe", 2 * QP)
        G.tensor_scalar(albe[:, 0:QP], umup[:, 0:QP], -1.0, 1.0,
                        op0=ALU.mult, op1=ALU.add)          # alp = 1 - Te*thm
        G.tensor_scalar_sub(albe[:, QP : 2 * QP], umup[:, QP : 2 * QP], 1.0)
        den1 = ptile("den1", 2 * QP)
        V.tensor_scalar(den1, e1, -1.0, 1.0, op0=ALU.mult, op1=ALU.add)
        V.reciprocal_approx_fast(den1, den1)                # 1/(1-e1)
        numL = ptile("numL", 2 * QP)
        G.tensor_scalar(numL, eL, -1.0, 1.0, op0=ALU.mult, op1=ALU.add)
        SmSp = ptile("SmSp", 2 * QP)
        V.tensor_mul(SmSp, numL, den1)                      # (Sm, Sp)
        Sm = SmSp[:, 0:QP]; Sp = SmSp[:, QP : 2 * QP]
        asbs = ptile("asbs", 2 * QP)
        V.tensor_mul(asbs, albe, SmSp)                      # (alp*Sm, bet*Sp)
        dsab = ptile("dsab", 2 * QP)
        V.tensor_sub(dsab[:, 0:QP], Sm, Sp)
        V.tensor_add(dsab[:, QP : 2 * QP], asbs[:, 0:QP], asbs[:, QP : 2 * QP])
        rr2 = ptile("rr2", 2 * QP)
        V.reciprocal_approx_fast(rr2, dsab)                 # (1/dS, 1/ab)
        rdS = rr2[:, 0:QP]; rab = rr2[:, QP : 2 * QP]
        w1 = ptile("w1"); G.tensor_mul(w1, vp, albe[:, 0:QP])     # off-chain
        w2 = ptile("w2"); G.tensor_mul(w2, vp, albe[:, QP : 2 * QP])
        vede = ptile("vede"); V.tensor_mul(vede, ve, rdS)
        w1b = ptile("w1b"); V.tensor_mul(w1b, w1, rab)
        w2b = ptile("w2b"); V.tensor_mul(w2b, w2, rab)
        V.tensor_add(c1c2[:, 0:QP], w1b, vede)              # c1 (fp16)
        V.tensor_sub(c1c2[:, QP : 2 * QP], w2b, vede)       # c2 (fp16)

        # c rows to DRAM (fp16) for per-pair broadcast DMAs
        crows_d = dpool.tile([4, HALF], f16, tag="crows_d", name="crows_d")
        nc.sync.dma_start(out=crows_d[0:2, :], in_=c1c2[:, 0:QP])
        nc.sync.dma_start(out=crows_d[2:4, :], in_=c1c2[:, QP : 2 * QP])

        # ---- main loop over pixel pairs ----
        tanh_scale = float(a_ / 2.0)
        fs1 = float(K2 / 2.0)
        fs2 = float(K1 + K2 / 2.0)

        def cb_fetch(p):
            lo = p * PW
            Wp = min(PW, HALF - lo)
            cb = cbp.tile([128, PW], f16, tag="cb", name="cb")
            c0 = crows_d[0, lo : lo + Wp]
            nc.sync.dma_start(
                out=cb[:, :Wp],
                in_=bass.AP(
                    tensor=c0.tensor, offset=c0.offset,
                    ap=[[HALF, 4], [0, 32], [1, Wp]],
                ),
            )
            return cb

        cbs = {p: cb_fetch(p) for p in range(min(3, NP))}

        for p in range(NP):
            lo = p * PW
            Wp = min(PW, HALF - lo)
            sl = slice(lo, lo + Wp)

            th_bc = psbc.tile([128, PW], f32, tag="th_bc", name="th_bc")
            for mlo in range(0, Wp, 512):
                mw = min(512, Wp - mlo)
                nc.tensor.matmul(
                    th_bc[:, mlo : mlo + mw], o4_r[:],
                    rowsT[:, lo + mlo : lo + mlo + mw].bitcast(f32r),
                    start=True, stop=True,
                )
            eb = ebp.tile([128, PW], f16, tag="eb", name="eb")
            nc.scalar.activation(
                eb[:, :Wp], th_bc[:, :Wp], AF.Exp, bias=0.0, scale=sv_sb[:, 0:1]
            )
            cb = cbs.pop(p)
            rhs = rhp.tile([128, PW], f16, tag="rhs", name="rhs")
            V.tensor_mul(rhs[:, :Wp], eb[:, :Wp], cb[:, :Wp])

            conc = pcc.tile([2 * TS, PW], f32, tag="conc", name="conc")
            for mlo in range(0, Wp, 512):
                mw = min(512, Wp - mlo)
                nc.tensor.matmul(
                    conc[:, mlo : mlo + mw], b4_sb[:],
                    rhs[:, mlo : mlo + mw], start=True, stop=True,
                )

            tht = ep1.tile([2 * TS, PW], f32, tag="tht", name="tht")
            nc.scalar.activation(
                tht[:, :Wp], conc[:, :Wp], AF.Tanh,
                bias=tb_sb, scale=tanh_scale,
            )
            rt = ep2.tile([2 * TS, PW], f32, tag="rt", name="rt")
            V.reciprocal_approx_fast(rt[:, :Wp], tht[:, :Wp])
            ob = obp.tile([2 * TS, PW], f16, tag="ob", name="ob")
            G.tensor_scalar(
                ob[:, :Wp], rt[:, :Wp], fs1, fs2, op0=ALU.mult, op1=ALU.add
            )
            nc.sync.dma_start(out=sig2[:, sl], in_=ob[:, :Wp])
            if p + 3 < NP:
                cbs[p + 3] = cb_fetch(p + 3)

    nc.compile()
    return nc


def _host_prep(sample_time: np.ndarray, Cp: np.ndarray):
    """Build the AIF response matrix A, fit the J-term exponential basis, and
    pack the block lhsT / broadcast-ones / scale constants."""
    t_end = float(np.asarray(sample_time)[-1])
    Lf = int(round(t_end / STEP)) + 1
    t_samp = np.arange(Lf, dtype=np.float32) * np.float32(STEP)
    aifci = np.interp(
        t_samp.astype(np.float64),
        np.asarray(sample_time, np.float64),
        np.asarray(Cp, np.float64),
    ).astype(np.float32)
    aif = np.concatenate([np.zeros(DELAY, np.float32), aifci[:-DELAY]])
    idx = np.searchsorted(t_samp, np.asarray(sample_time, np.float32), side="left")
    idx = np.minimum(idx, Lf - 1)

    # A[k, t] = aif[idx[k] - t] for t <= idx[k] (conv 'full' sampled at idx)
    A = np.zeros((TS, 640), np.float64)
    for k in range(TS):
        i = int(idx[k])
        A[k, : i + 1] = aif[i::-1]

    alphas = _alphas()
    tg = np.arange(640, dtype=np.float64)
    g = np.geomspace(0.02, 64.0, 4000)
    P = np.exp(-0.1 * np.outer(g, tg)) @ A.T          # [G, 50]
    M = np.exp(-np.outer(g, alphas))                  # [G, J]
    B = np.linalg.solve(M.T @ M + 1e-8 * np.eye(J), M.T @ P)  # [J, 50]
    B = B.astype(np.float32)

    # block lhsT [128, 100]: blocks (em h0, em h1, ep h0, ep h1) x 32 alphas
    # (rows tensors are (thm h0, thm h1, thp h0, thp h1) so that each rows
    # quantity ships in one [128,100] -> [2,6400] reshape DMA)
    b4 = np.zeros((128, 2 * TS), np.float32)
    b4[0:J, 0:TS] = B
    b4[J : 2 * J, TS : 2 * TS] = B
    b4[2 * J : 3 * J, 0:TS] = B
    b4[3 * J : 4 * J, TS : 2 * TS] = B

    ones4 = np.zeros((4, 128), np.float32)
    for r in range(4):
        ones4[r, r * J : (r + 1) * J] = 1.0

    sv = (-alphas[np.arange(128) % J]).reshape(128, 1).astype(np.float32)
    return b4.astype(np.float16), ones4, sv


def kernel(param: np.ndarray, sample_time: np.ndarray, Cp: np.ndarray) -> np.ndarray:
    from concourse.bass_utils import run_bass_kernel_spmd

    if "nc" not in _CACHE:
        _CACHE["nc"] = _build_bass()
    nc = _CACHE["nc"]

    b4, ones4, sv = _host_prep(sample_time, Cp)
    pflat = np.ascontiguousarray(np.asarray(param, np.float32).reshape(4, NPIX))
    in_maps = []
    for c in range(NCORES):
        in_maps.append(
            {
                "pmap": np.ascontiguousarray(pflat[:, c * SHARD : (c + 1) * SHARD]),
                "b4": b4,
                "ones4": ones4,
                "sv": sv,
            }
        )
    res = run_bass_kernel_spmd(
        nc,
        in_maps,
        core_ids=list(range(NCORES)),
        trace=bool(int(os.environ.get("DCE_TRACE", "0"))),
    )
    if res.exec_time_ns is not None:
        _CACHE["exec_time_ns"] = res.exec_time_ns
    # sig2 [100, 6400] fp16 per core: rows 0-49 half0 pixels, rows 50-99 half1
    parts = []
    for r in res.results:
        s2 = np.asarray(r["sig2"], np.float32)
        parts.append(np.concatenate([s2[:TS, :], s2[TS:, :]], axis=1))
    out = np.concatenate(parts, axis=1)
    return out.reshape(TS, 1, H, W)
